# revision 18
# baseline (speedup 1.0000x reference)
"""BuildGraphPyramid kernel for Trainium2 (8 NeuronCores).

Pipeline per batch (B=4): FPS 8192->2048->512, then 7 KNN(k=16) problems.
Cores 2b and 2b+1 both handle batch b: each runs FPS redundantly (it is a
sequential chain), then they split the two largest KNNs (queries = level-0
points) by half via a host-provided query-point input; the smaller KNNs are
computed redundantly on both cores (v1).

Layouts:
  - level points in SBUF as (128, F, 3) with point n = p*F + f
  - per-level support tensor rhs4 (4, N): rows = [2x; 2y; 2z; -(x^2+y^2+z^2)],
    columns in natural point order. A KNN tile computes
    psum = qT4_chunk^T @ rhs4_chunk = 2 q.s - |s|^2 (+ q_row const 1 * -s2)
    which ranks support points identically to -(squared distance).
  - top-16 per query row via vector-engine max8 / match_replace / max_index.
"""
import sys

sys.path.insert(0, "/opt/trn_rl_repo")
import numpy as np

B, N0, N1, N2, K = 4, 8192, 2048, 512, 16
P = 128
NCORES = 8
NEG = -3.0e38

_cache = {}


def _patch_tile_drain():
    """This walrus build cannot encode >1 sem wait on the TPB_CTRL drain that
    TileContext emits at exit; split the extra waits onto chained nops."""
    from concourse.tile import TileContext
    from concourse.vector_clock import ScopedClock
    import concourse.mybir as mybir

    if getattr(TileContext, "_drain_patched", False):
        return

    def patched(self, tick_clock, wait_clock):
        nc = self.nc
        drain_inst = nc.sync.drain()
        wait_clock.add_sem_waits(
            drain_inst.ins, ScopedClock({None: tick_clock.global_clock})
        )
        si = drain_inst.ins.sync_info
        if si is not None and si.on_wait and len(si.on_wait) > 1:
            waits = list(si.on_wait)
            si.on_wait = [waits[0]]
            for w in waits[1:]:
                nop = nc.sync.nop(nofuse=True, hint="drain_wait_split")
                nsi = nop.ins.sync_info
                if nsi is None:
                    nop.ins.sync_info = mybir.SyncInfo(on_wait=[w], on_update=[])
                else:
                    nsi.on_wait = list(nsi.on_wait) + [w]
        nc.all_engine_barrier()
        assert self.sems is not None
        popped = nc._tile_sem_poison_stack.pop()
        assert popped is self._sem_poison
        nc.clear_and_free_semaphores(list(self.sems.allocated().values()))
        nc.all_engine_barrier()

    TileContext._drain_and_barrier = patched
    TileContext._drain_patched = True


def _split_ctrl_waits(nc):
    """This walrus build cannot encode >1 sem wait on TPB_CTRL-lowered
    instructions (Drain/NoOp/EventSemaphore/branches). Keep one wait on the
    instruction and hoist the rest onto same-engine NoOps inserted before."""
    import concourse.mybir as mybir

    ctrl = (
        mybir.InstDrain, mybir.InstNoOp, mybir.InstEventSemaphore,
        mybir.InstUnconditionalBranch, mybir.InstCompareAndBranch,
    )
    for fn in nc.m.functions:
        for blk in fn.blocks:
            insts = list(blk.instructions)
            out = []
            changed = False
            for inst in insts:
                si = inst.sync_info
                cap = 1
                if si is not None and si.on_wait and len(si.on_wait) > cap:
                    waits = list(si.on_wait)
                    si.on_wait = waits[:cap]
                    inst.sync_info = si
                    for w in waits[cap:]:
                        nop = mybir.InstNoOp(
                            name=nc.get_next_instruction_name(),
                            engine=inst.engine,
                            bass_nofuse=True,
                            sync_info=mybir.SyncInfo(on_wait=[w], on_update=[]),
                        )
                        nc.register_instruction(nop)
                        out.append(nop)
                    changed = True
                out.append(inst)
            if changed:
                blk.instructions = out


def _build_program(fps_unroll=8):
    import contextlib

    import concourse.bass as bass
    import concourse.mybir as mybir
    from concourse.bass import ds
    from concourse.masks import make_identity
    from concourse.tile import TileContext

    _patch_tile_drain()

    f32 = mybir.dt.float32
    i32 = mybir.dt.int32
    u16 = mybir.dt.uint16
    AF = mybir.ActivationFunctionType
    OP = mybir.AluOpType

    nc = bass.Bass()
    pts_d = nc.declare_dram_parameter("pts", [N0, 3], f32, isOutput=False)
    qh0_d = nc.declare_dram_parameter("qh0", [N0 // 2, 3], f32, isOutput=False)
    l1o = nc.declare_dram_parameter("l1o", [N1, 3], f32, isOutput=True)
    l2o = nc.declare_dram_parameter("l2o", [N2, 3], f32, isOutput=True)
    n0o = nc.declare_dram_parameter("n0o", [N0 // 2, K], i32, isOutput=True)
    u0o = nc.declare_dram_parameter("u0o", [N0 // 2, K], i32, isOutput=True)
    s0o = nc.declare_dram_parameter("s0o", [N1, K], i32, isOutput=True)
    n1o = nc.declare_dram_parameter("n1o", [N1, K], i32, isOutput=True)
    u1o = nc.declare_dram_parameter("u1o", [N1, K], i32, isOutput=True)
    s1o = nc.declare_dram_parameter("s1o", [N2, K], i32, isOutput=True)
    n2o = nc.declare_dram_parameter("n2o", [N2, K], i32, isOutput=True)

    with TileContext(nc) as tc, contextlib.ExitStack() as ctx:
        consts = ctx.enter_context(tc.tile_pool(name="consts", bufs=1))
        persist = ctx.enter_context(tc.tile_pool(name="persist", bufs=1))
        fps_ctx = contextlib.ExitStack()
        fps_psum = fps_ctx.enter_context(tc.tile_pool(name="fps_psum", bufs=3, space="PSUM"))
        fps_scr = fps_ctx.enter_context(tc.tile_pool(name="fps_scr", bufs=3))

        ident = consts.tile([P, P], f32)
        make_identity(nc, ident)
        ones3 = consts.tile([3, 1], f32)
        nc.vector.memset(ones3, 1.0)
        ones_row = consts.tile([1, 512], f32)
        nc.vector.memset(ones_row, 1.0)
        ones_col = consts.tile([P, 1], f32)
        nc.vector.memset(ones_col, 1.0)
        onesr128 = consts.tile([1, P], f32)
        nc.vector.memset(onesr128, 1.0)
        negones_col = consts.tile([P, 1], f32)
        nc.vector.memset(negones_col, -1.0)

        def fill_ones(dst, N):
            for c in range(0, N, 512):
                nc.sync.dma_start(dst[0:1, c : c + 512], ones_row[:, : min(512, N - c)])

        # ------------- load level-0 points -------------
        pts_sb = persist.tile([P, N0 // P, 3], f32)
        nc.sync.dma_start(pts_sb, pts_d.rearrange("(p f) c -> p f c", p=P))

        # ---------------- FPS ----------------
        def fps(src_sb, F, m, out_dram):
            """src_sb: (P, F, 3) points, n = p*F + f. Writes m selected points
            (in selection order) to out_dram (m, 3). Selection matches
            reference _fps_single bit-exactly (modulo exact-tie argmax)."""
            px = persist.tile([P, F], f32, tag=f"px{F}")
            py = persist.tile([P, F], f32, tag=f"py{F}")
            pz = persist.tile([P, F], f32, tag=f"pz{F}")
            nc.vector.tensor_copy(px, src_sb[:, :, 0])
            nc.vector.tensor_copy(py, src_sb[:, :, 1])
            nc.vector.tensor_copy(pz, src_sb[:, :, 2])

            mind = persist.tile([P, F], f32, tag=f"mind{F}")
            bneg = persist.tile([P, 3], f32, tag=f"bneg{F}")
            nc.vector.memset(mind, float("inf"))

            def extract(oh, outbuf):
                """oh: one-hot (P,F). Computes bneg = -coords (all partitions)
                and writes +coords (1,3) into outbuf (an SBUF slice)."""
                prod3 = fps_scr.tile([P, F, 3], f32, tag=f"prod{F}")
                nc.vector.tensor_tensor(
                    out=prod3, in0=oh[:, :, None].broadcast_to([P, F, 3]),
                    in1=src_sb, op=OP.mult)
                creg = fps_scr.tile([P, 3], f32, tag=f"cneg{F}")
                nc.vector.tensor_reduce(
                    creg[:, :, None], prod3.rearrange("p f c -> p c f"),
                    mybir.AxisListType.X, OP.add)
                # cross-partition sum of the single nonzero row -> (1,3), negated
                psum1 = fps_psum.tile([P, P], f32, tag="fps_ps")
                nc.tensor.matmul(psum1[0:1, 0:3], negones_col, creg, start=True, stop=True)
                xyzneg = fps_scr.tile([1, 3], f32, tag=f"xn{F}")
                nc.scalar.activation(xyzneg, psum1[0:1, 0:3], AF.Copy)
                nc.vector.tensor_scalar_mul(outbuf, xyzneg, -1.0)
                # broadcast -coords to all partitions for the next step's bias
                psumb = fps_psum.tile([P, P], f32, tag="fps_ps")
                nc.tensor.matmul(psumb[:, 0:3], onesr128, xyzneg, start=True, stop=True)
                nc.scalar.activation(bneg, psumb[:, 0:3], AF.Copy)

            def select_next(oh_out):
                """One FPS step: distances to last point, min-update, global
                argmax -> one-hot in oh_out."""
                sqx = fps_scr.tile([P, F], f32, tag=f"sqx{F}")
                sqy = fps_scr.tile([P, F], f32, tag=f"sqy{F}")
                sqz = fps_scr.tile([P, F], f32, tag=f"sqz{F}")
                nc.scalar.activation(sqx, px, AF.Square, bias=bneg[:, 0:1], scale=1.0)
                nc.scalar.activation(sqy, py, AF.Square, bias=bneg[:, 1:2], scale=1.0)
                nc.scalar.activation(sqz, pz, AF.Square, bias=bneg[:, 2:3], scale=1.0)
                nc.vector.tensor_add(sqx, sqx, sqy)
                nc.vector.tensor_add(sqx, sqx, sqz)
                mp = fps_scr.tile([P, 1], f32, tag=f"mp{F}")
                nc.vector.tensor_tensor(out=mind, in0=mind, in1=sqx, op=OP.min)
                nc.vector.tensor_reduce(mp, mind, mybir.AxisListType.X, OP.max)
                pst = fps_psum.tile([P, P], f32, tag="fps_ps")
                nc.tensor.transpose(pst[0:1, :], mp, ident)
                rm = fps_scr.tile([1, P], f32, tag=f"rm{F}")
                nc.scalar.activation(rm, pst[0:1, :], AF.Copy)
                g8 = fps_scr.tile([1, 8], f32, tag=f"g8{F}")
                nc.vector.max(out=g8, in_=rm)
                psM = fps_psum.tile([P, P], f32, tag="fps_ps")
                nc.tensor.matmul(psM[:, 0:1], onesr128, g8[0:1, 0:1], start=True, stop=True)
                Mb = fps_scr.tile([P, 1], f32, tag=f"Mb{F}")
                nc.scalar.activation(Mb, psM[:, 0:1], AF.Copy)
                nc.vector.tensor_scalar(
                    out=oh_out, in0=mind, scalar1=Mb[:, 0:1], scalar2=None,
                    op0=OP.is_equal)

            oh0 = fps_scr.tile([P, F], f32, tag=f"oh{F}")
            nc.vector.memset(oh0, 0.0)
            nc.vector.memset(oh0[0:1, 0:1], 1.0)
            xyz0 = fps_scr.tile([1, 3], f32, tag=f"xyz{F}")
            extract(oh0, xyz0)
            nc.sync.dma_start(out_dram[0:1, :], xyz0)

            G = fps_unroll
            n_grp = (m - 1) // G
            full_end = 1 + n_grp * G

            def group_body(t0, grp):
                xyzbuf = fps_scr.tile([1, 3 * grp], f32, tag=f"xyzb{F}")
                for u in range(grp):
                    oh = fps_scr.tile([P, F], f32, tag=f"oh{F}")
                    select_next(oh)
                    extract(oh, xyzbuf[0:1, 3 * u : 3 * u + 3])
                if isinstance(t0, int):
                    nc.sync.dma_start(
                        out_dram[t0 : t0 + grp, :], xyzbuf)
                else:
                    nc.sync.dma_start(out_dram[ds(t0, grp), :], xyzbuf)

            if n_grp > 0:
                with tc.For_i(1, full_end, G) as t0:
                    group_body(t0, G)
            if full_end < m:
                group_body(full_end, m - full_end)

        fps(pts_sb, N0 // P, N1, l1o)
        tc.strict_bb_all_engine_barrier()

        pts2_sb = persist.tile([P, N1 // P, 3], f32)
        nc.sync.dma_start(pts2_sb, l1o.rearrange("(p f) c -> p f c", p=P))
        fps(pts2_sb, N1 // P, N2, l2o)
        tc.strict_bb_all_engine_barrier()

        pts3_sb = persist.tile([P, N2 // P, 3], f32)
        nc.sync.dma_start(pts3_sb, l2o.rearrange("(p f) c -> p f c", p=P))

        fps_ctx.close()

        # ---------------- KNN support/query construction ----------------
        build_ctx = contextlib.ExitStack()
        psum_t = build_ctx.enter_context(tc.tile_pool(name="psum_t", bufs=2, space="PSUM"))
        psum_s = build_ctx.enter_context(tc.tile_pool(name="psum_s", bufs=2, space="PSUM"))
        build_scr = build_ctx.enter_context(tc.tile_pool(name="build_scr", bufs=2))

        def build_rhs4(src_sb, N):
            """src_sb (P, F, 3) -> rhs4 (4, N): [2x; 2y; 2z; -|s|^2], natural
            column order n = p*F + f."""
            F = N // P
            rhs4 = persist.tile([4, N], f32, tag=f"rhs4_{N}")
            r3 = rhs4.rearrange("k (p f) -> k f p", p=P)
            for j in range(F):
                pst = psum_t.tile([P, P], f32, tag="tp")
                nc.tensor.transpose(pst[0:3, :], src_sb[:, j, :], ident)
                nc.scalar.activation(r3[0:3, j, :], pst[0:3, :], AF.Copy, scale=2.0)
            for c in range(0, N, 512):
                sq3c = build_scr.tile([3, 512], f32, tag="sq3c")
                nc.scalar.activation(sq3c, rhs4[0:3, c : c + 512], AF.Square)
                ps2 = psum_s.tile([1, 512], f32, tag="s2")
                nc.tensor.matmul(ps2, ones3, sq3c, start=True, stop=True)
                # compute-engine writes must start at partition 0; stage the
                # -|s|^2 row there and DMA it into partition 3 of rhs4.
                s2c = build_scr.tile([1, 512], f32, tag="s2c")
                nc.scalar.activation(s2c, ps2, AF.Copy, scale=-0.25)
                nc.sync.dma_start(rhs4[3:4, c : c + 512], s2c)
            return rhs4

        def qt4_from_rhs4(rhs4, N):
            qt4 = persist.tile([4, N], f32, tag=f"qt4_{N}")
            nc.scalar.activation(qt4[0:3, :], rhs4[0:3, :], AF.Copy, scale=0.5)
            fill_ones(qt4[3:4, :], N)
            return qt4

        rhs4_0 = build_rhs4(pts_sb, N0)
        rhs4_1 = build_rhs4(pts2_sb, N1)
        rhs4_2 = build_rhs4(pts3_sb, N2)
        qt4_1 = qt4_from_rhs4(rhs4_1, N1)
        qt4_2 = qt4_from_rhs4(rhs4_2, N2)

        # level-0 query half from host input
        qh_sb = build_scr.tile([P, N0 // 2 // P, 3], f32)
        nc.sync.dma_start(qh_sb, qh0_d.rearrange("(p f) c -> p f c", p=P))
        qt4_0 = persist.tile([4, N0 // 2], f32, tag="qt4_0")
        q3 = qt4_0.rearrange("k (p f) -> k f p", p=P)
        for j in range(N0 // 2 // P):
            pst = psum_t.tile([P, P], f32, tag="tp")
            nc.tensor.transpose(pst[0:3, :], qh_sb[:, j, :], ident)
            nc.scalar.activation(q3[0:3, j, :], pst[0:3, :], AF.Copy, scale=1.0)
        fill_ones(qt4_0[3:4, :], N0 // 2)

        build_ctx.close()

        # ---------------- KNN compute ----------------
        knn_pool = ctx.enter_context(tc.tile_pool(name="knn", bufs=2))
        d2_pool = ctx.enter_context(tc.tile_pool(name="knn_d2", bufs=1))
        psum_mm = ctx.enter_context(tc.tile_pool(name="psum_mm", bufs=4, space="PSUM"))
        idx_pool = ctx.enter_context(tc.tile_pool(name="idx", bufs=3))

        def chunk_max8(dst8, src, Ns, nchunk):
            """Per-512-chunk top-8 -> (P, nchunk*8) candidates; then global
            top-8 of candidates -> dst8. Exact for the top-8 of src."""
            if nchunk == 1:
                nc.vector.max(out=dst8, in_=src)
                return
            cands = idx_pool.tile([P, nchunk * 8], f32, tag="cands")
            for c in range(nchunk):
                nc.vector.max(
                    out=cands[:, c * 8 : (c + 1) * 8],
                    in_=src[:, c * 512 : min((c + 1) * 512, Ns)])
            nc.vector.max(out=dst8, in_=cands)

        def knn(qt4, Nq, rhs4, Ns, out_dram):
            nqt = Nq // P
            nchunk = (Ns + 511) // 512
            for j in range(nqt):
                lhsT = qt4[:, j * P : (j + 1) * P]
                dtile = knn_pool.tile([P, Ns], f32, tag="dtile")
                for c in range(nchunk):
                    cw = min(512, Ns - c * 512)
                    pmm = psum_mm.tile([P, 512], f32, tag="mm")
                    nc.tensor.matmul(
                        pmm[:, :cw], lhsT, rhs4[:, c * 512 : c * 512 + cw],
                        start=True, stop=True)
                    nc.scalar.activation(
                        dtile[:, c * 512 : c * 512 + cw], pmm[:, :cw], AF.Copy)
                # exact top-16: top-8, knock them out of a copy, top-8 again
                v1 = idx_pool.tile([P, 8], f32, tag="v1")
                v2 = idx_pool.tile([P, 8], f32, tag="v2")
                chunk_max8(v1, dtile, Ns, nchunk)
                d2 = d2_pool.tile([P, Ns], f32, tag="d2")
                nc.vector.match_replace(
                    out=d2, in_to_replace=v1, in_values=dtile, imm_value=NEG)
                chunk_max8(v2, d2, Ns, nchunk)
                i1 = idx_pool.tile([P, 8], u16, tag="i1")
                i2 = idx_pool.tile([P, 8], u16, tag="i2")
                nc.vector.max_index(out=i1, in_max=v1, in_values=dtile)
                nc.vector.max_index(out=i2, in_max=v2, in_values=d2)
                ii = idx_pool.tile([P, K], i32, tag="ii")
                nc.vector.tensor_copy(ii[:, 0:8], i1)
                nc.vector.tensor_copy(ii[:, 8:16], i2)
                nc.sync.dma_start(out_dram[j * P : (j + 1) * P, :], ii)

        knn(qt4_0, N0 // 2, rhs4_0, N0, n0o)   # knn0 half
        knn(qt4_0, N0 // 2, rhs4_1, N1, u0o)   # up0 half
        knn(qt4_1, N1, rhs4_0, N0, s0o)        # sub0 full
        knn(qt4_1, N1, rhs4_1, N1, n1o)        # knn1 full
        knn(qt4_1, N1, rhs4_2, N2, u1o)        # up1 full
        knn(qt4_2, N2, rhs4_1, N1, s1o)        # sub1 full
        knn(qt4_2, N2, rhs4_2, N2, n2o)        # knn2 full

    _split_ctrl_waits(nc)
    return nc


def _get_program():
    if "nc" not in _cache:
        _cache["nc"] = _build_program()
    return _cache["nc"]


def kernel(points: np.ndarray):
    from concourse.bass_utils import run_bass_kernel_spmd

    points = np.asarray(points)
    assert points.shape == (B, N0, 3)
    nc = _get_program()

    in_maps = []
    for core in range(NCORES):
        b, h = core // 2, core % 2
        pts = np.ascontiguousarray(points[b], dtype=np.float32)
        qh0 = np.ascontiguousarray(
            points[b, h * (N0 // 2) : (h + 1) * (N0 // 2)], dtype=np.float32)
        in_maps.append({"pts": pts, "qh0": qh0})

    res = run_bass_kernel_spmd(nc, in_maps, list(range(NCORES)))
    r = res.results

    l1 = np.stack([r[2 * b]["l1o"] for b in range(B)])
    l2 = np.stack([r[2 * b]["l2o"] for b in range(B)])
    n0 = np.stack(
        [np.concatenate([r[2 * b]["n0o"], r[2 * b + 1]["n0o"]], 0) for b in range(B)])
    u0 = np.stack(
        [np.concatenate([r[2 * b]["u0o"], r[2 * b + 1]["u0o"]], 0) for b in range(B)])
    s0 = np.stack([r[2 * b]["s0o"] for b in range(B)])
    n1 = np.stack([r[2 * b]["n1o"] for b in range(B)])
    u1 = np.stack([r[2 * b]["u1o"] for b in range(B)])
    s1 = np.stack([r[2 * b]["s1o"] for b in range(B)])
    n2 = np.stack([r[2 * b]["n2o"] for b in range(B)])

    return (
        points.astype(np.float32), l1, l2,
        n0.astype(np.int32), n1.astype(np.int32), n2.astype(np.int32),
        s0.astype(np.int32), s1.astype(np.int32),
        u0.astype(np.int32), u1.astype(np.int32),
    )


# revision 23
# speedup vs baseline: 1.7174x; 1.7174x over previous
"""BuildGraphPyramid kernel for Trainium2 (8 NeuronCores).

Pipeline per batch (B=4): FPS 8192->2048->512, then 7 KNN(k=16) problems.
Cores 2b and 2b+1 both handle batch b: each runs FPS redundantly (it is a
sequential chain), then they split the two largest KNNs (queries = level-0
points) by half via a host-provided query-point input; the smaller KNNs are
computed redundantly on both cores (v1).

Layouts:
  - level points in SBUF as (128, F, 3) with point n = p*F + f
  - per-level support tensor rhs4 (4, N): rows = [2x; 2y; 2z; -(x^2+y^2+z^2)],
    columns in natural point order. A KNN tile computes
    psum = qT4_chunk^T @ rhs4_chunk = 2 q.s - |s|^2 (+ q_row const 1 * -s2)
    which ranks support points identically to -(squared distance).
  - top-16 per query row via vector-engine max8 / match_replace / max_index.
"""
import sys

sys.path.insert(0, "/opt/trn_rl_repo")
import numpy as np

B, N0, N1, N2, K = 4, 8192, 2048, 512, 16
P = 128
NCORES = 8
NEG = -3.0e38

_cache = {}

# output blob layout: name -> (offset in int32 elements, rows, cols)
OLAYOUT = {
    "n0o": (0, N0 // 2, K),
    "u0o": (65536, N0 // 2, K),
    "s0o": (131072, N1, K),
    "n1o": (163840, N1, K),
    "u1o": (196608, N1, K),
    "s1o": (229376, N2, K),
    "n2o": (237568, N2, K),
    "l1o": (245760, N1, 3),
    "l2o": (251904, N2, 3),
}
OTOTAL = 253440


def _patch_tile_drain():
    """This walrus build cannot encode >1 sem wait on the TPB_CTRL drain that
    TileContext emits at exit; split the extra waits onto chained nops."""
    from concourse.tile import TileContext
    from concourse.vector_clock import ScopedClock
    import concourse.mybir as mybir

    if getattr(TileContext, "_drain_patched", False):
        return

    def patched(self, tick_clock, wait_clock):
        nc = self.nc
        drain_inst = nc.sync.drain()
        wait_clock.add_sem_waits(
            drain_inst.ins, ScopedClock({None: tick_clock.global_clock})
        )
        si = drain_inst.ins.sync_info
        if si is not None and si.on_wait and len(si.on_wait) > 1:
            waits = list(si.on_wait)
            si.on_wait = [waits[0]]
            for w in waits[1:]:
                nop = nc.sync.nop(nofuse=True, hint="drain_wait_split")
                nsi = nop.ins.sync_info
                if nsi is None:
                    nop.ins.sync_info = mybir.SyncInfo(on_wait=[w], on_update=[])
                else:
                    nsi.on_wait = list(nsi.on_wait) + [w]
        nc.all_engine_barrier()
        assert self.sems is not None
        popped = nc._tile_sem_poison_stack.pop()
        assert popped is self._sem_poison
        nc.clear_and_free_semaphores(list(self.sems.allocated().values()))
        nc.all_engine_barrier()

    TileContext._drain_and_barrier = patched
    TileContext._drain_patched = True


def _split_ctrl_waits(nc):
    """This walrus build cannot encode >1 sem wait on TPB_CTRL-lowered
    instructions (Drain/NoOp/EventSemaphore/branches). Keep one wait on the
    instruction and hoist the rest onto same-engine NoOps inserted before."""
    import concourse.mybir as mybir

    ctrl = (
        mybir.InstDrain, mybir.InstNoOp, mybir.InstEventSemaphore,
        mybir.InstUnconditionalBranch, mybir.InstCompareAndBranch,
    )
    for fn in nc.m.functions:
        for blk in fn.blocks:
            insts = list(blk.instructions)
            out = []
            changed = False
            for inst in insts:
                si = inst.sync_info
                cap = 1
                if si is not None and si.on_wait and len(si.on_wait) > cap:
                    waits = list(si.on_wait)
                    si.on_wait = waits[:cap]
                    inst.sync_info = si
                    for w in waits[cap:]:
                        nop = mybir.InstNoOp(
                            name=nc.get_next_instruction_name(),
                            engine=inst.engine,
                            bass_nofuse=True,
                            sync_info=mybir.SyncInfo(on_wait=[w], on_update=[]),
                        )
                        nc.register_instruction(nop)
                        out.append(nop)
                    changed = True
                out.append(inst)
            if changed:
                blk.instructions = out


def _build_program(fps_unroll=8, do_fps=True, do_knn=True, stage=99):
    import contextlib

    import concourse.bass as bass
    import concourse.mybir as mybir
    from concourse.bass import ds
    from concourse.masks import make_identity
    from concourse.tile import TileContext

    _patch_tile_drain()

    f32 = mybir.dt.float32
    i32 = mybir.dt.int32
    u16 = mybir.dt.uint16
    AF = mybir.ActivationFunctionType
    OP = mybir.AluOpType

    nc = bass.Bass()
    # Single input / single output blob: the PJRT-over-axon path pays a large
    # fixed cost per I/O *tensor*, so everything is packed into two tensors.
    pin = nc.declare_dram_parameter("pin", [N0 + N0 // 2, 3], f32, isOutput=False)
    oall = nc.declare_dram_parameter("oall", [OTOTAL], i32, isOutput=True)
    pts_d = pin[0:N0, :]
    qh0_d = pin[N0 : N0 + N0 // 2, :]

    def oview(name):
        off, rows, cols = OLAYOUT[name]
        v = oall[off : off + rows * cols]
        if name in ("l1o", "l2o"):
            v = v.bitcast(f32)
        return v.rearrange("(r k) -> r k", k=cols)

    l1o = oview("l1o")
    l2o = oview("l2o")
    n0o = oview("n0o")
    u0o = oview("u0o")
    s0o = oview("s0o")
    n1o = oview("n1o")
    u1o = oview("u1o")
    s1o = oview("s1o")
    n2o = oview("n2o")

    with TileContext(nc) as tc, contextlib.ExitStack() as ctx:
        consts = ctx.enter_context(tc.tile_pool(name="consts", bufs=1))
        persist = ctx.enter_context(tc.tile_pool(name="persist", bufs=1))
        fps_ctx = contextlib.ExitStack()
        fps_psum = fps_ctx.enter_context(tc.tile_pool(name="fps_psum", bufs=3, space="PSUM"))
        fps_scr = fps_ctx.enter_context(tc.tile_pool(name="fps_scr", bufs=3))

        ident = consts.tile([P, P], f32)
        make_identity(nc, ident)
        ones3 = consts.tile([3, 1], f32)
        nc.vector.memset(ones3, 1.0)
        ones_row = consts.tile([1, 512], f32)
        nc.vector.memset(ones_row, 1.0)
        ones_col = consts.tile([P, 1], f32)
        nc.vector.memset(ones_col, 1.0)
        onesr128 = consts.tile([1, P], f32)
        nc.vector.memset(onesr128, 1.0)
        negones_col = consts.tile([P, 1], f32)
        nc.vector.memset(negones_col, -1.0)

        def fill_ones(dst, N):
            for c in range(0, N, 512):
                nc.sync.dma_start(dst[0:1, c : c + 512], ones_row[:, : min(512, N - c)])

        # ------------- load level-0 points -------------
        pts_sb = persist.tile([P, N0 // P, 3], f32)
        nc.sync.dma_start(pts_sb, pts_d.rearrange("(p f) c -> p f c", p=P))

        # ---------------- FPS ----------------
        def fps(src_sb, F, m, out_dram):
            """src_sb: (P, F, 3) points, n = p*F + f. Writes m selected points
            (in selection order) to out_dram (m, 3). Selection matches
            reference _fps_single bit-exactly (modulo exact-tie argmax)."""
            px = persist.tile([P, F], f32, tag=f"px{F}")
            py = persist.tile([P, F], f32, tag=f"py{F}")
            pz = persist.tile([P, F], f32, tag=f"pz{F}")
            nc.vector.tensor_copy(px, src_sb[:, :, 0])
            nc.vector.tensor_copy(py, src_sb[:, :, 1])
            nc.vector.tensor_copy(pz, src_sb[:, :, 2])

            mind = persist.tile([P, F], f32, tag=f"mind{F}")
            bneg = persist.tile([P, 3], f32, tag=f"bneg{F}")
            nc.vector.memset(mind, float("inf"))

            def extract(oh, outbuf):
                """oh: one-hot (P,F). Computes bneg = -coords (all partitions)
                and writes +coords (1,3) into outbuf (an SBUF slice)."""
                prod3 = fps_scr.tile([P, F, 3], f32, tag=f"prod{F}")
                nc.vector.tensor_tensor(
                    out=prod3, in0=oh[:, :, None].broadcast_to([P, F, 3]),
                    in1=src_sb, op=OP.mult)
                creg = fps_scr.tile([P, 3], f32, tag=f"cneg{F}")
                nc.vector.tensor_reduce(
                    creg[:, :, None], prod3.rearrange("p f c -> p c f"),
                    mybir.AxisListType.X, OP.add)
                # cross-partition sum of the single nonzero row -> (1,3), negated
                psum1 = fps_psum.tile([P, P], f32, tag="fps_ps")
                nc.tensor.matmul(psum1[0:1, 0:3], negones_col, creg, start=True, stop=True)
                xyzneg = fps_scr.tile([1, 3], f32, tag=f"xn{F}")
                nc.scalar.activation(xyzneg, psum1[0:1, 0:3], AF.Copy)
                nc.vector.tensor_scalar_mul(outbuf, xyzneg, -1.0)
                # broadcast -coords to all partitions for the next step's bias
                psumb = fps_psum.tile([P, P], f32, tag="fps_ps")
                nc.tensor.matmul(psumb[:, 0:3], onesr128, xyzneg, start=True, stop=True)
                nc.scalar.activation(bneg, psumb[:, 0:3], AF.Copy)

            def select_next(oh_out):
                """One FPS step: distances to last point, min-update, global
                argmax -> one-hot in oh_out."""
                sqx = fps_scr.tile([P, F], f32, tag=f"sqx{F}")
                sqy = fps_scr.tile([P, F], f32, tag=f"sqy{F}")
                sqz = fps_scr.tile([P, F], f32, tag=f"sqz{F}")
                nc.scalar.activation(sqx, px, AF.Square, bias=bneg[:, 0:1], scale=1.0)
                nc.scalar.activation(sqy, py, AF.Square, bias=bneg[:, 1:2], scale=1.0)
                nc.scalar.activation(sqz, pz, AF.Square, bias=bneg[:, 2:3], scale=1.0)
                nc.vector.tensor_add(sqx, sqx, sqy)
                nc.vector.tensor_add(sqx, sqx, sqz)
                mp = fps_scr.tile([P, 1], f32, tag=f"mp{F}")
                nc.vector.tensor_tensor(out=mind, in0=mind, in1=sqx, op=OP.min)
                nc.vector.tensor_reduce(mp, mind, mybir.AxisListType.X, OP.max)
                pst = fps_psum.tile([P, P], f32, tag="fps_ps")
                nc.tensor.transpose(pst[0:1, :], mp, ident)
                rm = fps_scr.tile([1, P], f32, tag=f"rm{F}")
                nc.scalar.activation(rm, pst[0:1, :], AF.Copy)
                g8 = fps_scr.tile([1, 8], f32, tag=f"g8{F}")
                nc.vector.max(out=g8, in_=rm)
                psM = fps_psum.tile([P, P], f32, tag="fps_ps")
                nc.tensor.matmul(psM[:, 0:1], onesr128, g8[0:1, 0:1], start=True, stop=True)
                Mb = fps_scr.tile([P, 1], f32, tag=f"Mb{F}")
                nc.scalar.activation(Mb, psM[:, 0:1], AF.Copy)
                nc.vector.tensor_scalar(
                    out=oh_out, in0=mind, scalar1=Mb[:, 0:1], scalar2=None,
                    op0=OP.is_equal)

            oh0 = fps_scr.tile([P, F], f32, tag=f"oh{F}")
            nc.vector.memset(oh0, 0.0)
            nc.vector.memset(oh0[0:1, 0:1], 1.0)
            xyz0 = fps_scr.tile([1, 3], f32, tag=f"xyz{F}")
            extract(oh0, xyz0)
            nc.sync.dma_start(out_dram[0:1, :], xyz0)

            G = fps_unroll
            n_grp = (m - 1) // G
            full_end = 1 + n_grp * G

            def group_body(t0, grp):
                xyzbuf = fps_scr.tile([1, 3 * grp], f32, tag=f"xyzb{F}")
                for u in range(grp):
                    oh = fps_scr.tile([P, F], f32, tag=f"oh{F}")
                    select_next(oh)
                    extract(oh, xyzbuf[0:1, 3 * u : 3 * u + 3])
                if isinstance(t0, int):
                    nc.sync.dma_start(
                        out_dram[t0 : t0 + grp, :], xyzbuf)
                else:
                    nc.sync.dma_start(out_dram[ds(t0, grp), :], xyzbuf)

            if n_grp > 0:
                with tc.For_i(1, full_end, G) as t0:
                    group_body(t0, G)
            if full_end < m:
                group_body(full_end, m - full_end)

        if do_fps:
            fps(pts_sb, N0 // P, N1, l1o)
        else:
            for r in range(0, N1, 512):
                nc.sync.dma_start(l1o[r : r + 512, :], pts_d[r : r + 512, :])
        tc.strict_bb_all_engine_barrier()

        pts2_sb = persist.tile([P, N1 // P, 3], f32)
        nc.sync.dma_start(pts2_sb, l1o.rearrange("(p f) c -> p f c", p=P))
        if do_fps:
            fps(pts2_sb, N1 // P, N2, l2o)
        else:
            nc.sync.dma_start(l2o[:, :], pts_d[0:N2, :])
        tc.strict_bb_all_engine_barrier()

        pts3_sb = persist.tile([P, N2 // P, 3], f32)
        nc.sync.dma_start(pts3_sb, l2o.rearrange("(p f) c -> p f c", p=P))

        fps_ctx.close()

        # ---------------- KNN support/query construction ----------------
        build_ctx = contextlib.ExitStack()
        psum_t = build_ctx.enter_context(tc.tile_pool(name="psum_t", bufs=2, space="PSUM"))
        psum_s = build_ctx.enter_context(tc.tile_pool(name="psum_s", bufs=2, space="PSUM"))
        build_scr = build_ctx.enter_context(tc.tile_pool(name="build_scr", bufs=2))

        def build_rhs4(src_sb, N):
            """src_sb (P, F, 3) -> rhs4 (4, N): [2x; 2y; 2z; -|s|^2], natural
            column order n = p*F + f."""
            F = N // P
            rhs4 = persist.tile([4, N], f32, tag=f"rhs4_{N}")
            r3 = rhs4.rearrange("k (p f) -> k f p", p=P)
            for j in range(F):
                pst = psum_t.tile([P, P], f32, tag="tp")
                nc.tensor.transpose(pst[0:3, :], src_sb[:, j, :], ident)
                nc.scalar.activation(r3[0:3, j, :], pst[0:3, :], AF.Copy, scale=2.0)
            for c in range(0, N, 512):
                sq3c = build_scr.tile([3, 512], f32, tag="sq3c")
                nc.scalar.activation(sq3c, rhs4[0:3, c : c + 512], AF.Square)
                ps2 = psum_s.tile([1, 512], f32, tag="s2")
                nc.tensor.matmul(ps2, ones3, sq3c, start=True, stop=True)
                # compute-engine writes must start at partition 0; stage the
                # -|s|^2 row there and DMA it into partition 3 of rhs4.
                s2c = build_scr.tile([1, 512], f32, tag="s2c")
                nc.scalar.activation(s2c, ps2, AF.Copy, scale=-0.25)
                nc.sync.dma_start(rhs4[3:4, c : c + 512], s2c)
            return rhs4

        def qt4_from_rhs4(rhs4, N):
            qt4 = persist.tile([4, N], f32, tag=f"qt4_{N}")
            nc.scalar.activation(qt4[0:3, :], rhs4[0:3, :], AF.Copy, scale=0.5)
            fill_ones(qt4[3:4, :], N)
            return qt4

        if stage >= 1:
            rhs4_0 = build_rhs4(pts_sb, N0)
            rhs4_1 = build_rhs4(pts2_sb, N1)
            rhs4_2 = build_rhs4(pts3_sb, N2)
        if stage >= 2:
            qt4_1 = qt4_from_rhs4(rhs4_1, N1)
            qt4_2 = qt4_from_rhs4(rhs4_2, N2)

        # level-0 query half from host input
        if stage < 3:
            do_knn_local = False
        qh_sb = build_scr.tile([P, N0 // 2 // P, 3], f32)
        nc.sync.dma_start(qh_sb, qh0_d.rearrange("(p f) c -> p f c", p=P))
        qt4_0 = persist.tile([4, N0 // 2], f32, tag="qt4_0")
        if stage >= 3:
            q3 = qt4_0.rearrange("k (p f) -> k f p", p=P)
            for j in range(N0 // 2 // P):
                pst = psum_t.tile([P, P], f32, tag="tp")
                nc.tensor.transpose(pst[0:3, :], qh_sb[:, j, :], ident)
                nc.scalar.activation(q3[0:3, j, :], pst[0:3, :], AF.Copy, scale=1.0)
            fill_ones(qt4_0[3:4, :], N0 // 2)

        build_ctx.close()

        # ---------------- KNN compute ----------------
        knn_pool = ctx.enter_context(tc.tile_pool(name="knn", bufs=2))
        d2_pool = ctx.enter_context(tc.tile_pool(name="knn_d2", bufs=1))
        psum_mm = ctx.enter_context(tc.tile_pool(name="psum_mm", bufs=4, space="PSUM"))
        idx_pool = ctx.enter_context(tc.tile_pool(name="idx", bufs=3))

        def chunk_max8(dst8, src, Ns, nchunk):
            """Per-512-chunk top-8 -> (P, nchunk*8) candidates; then global
            top-8 of candidates -> dst8. Exact for the top-8 of src."""
            if nchunk == 1:
                nc.vector.max(out=dst8, in_=src)
                return
            cands = idx_pool.tile([P, nchunk * 8], f32, tag="cands")
            for c in range(nchunk):
                nc.vector.max(
                    out=cands[:, c * 8 : (c + 1) * 8],
                    in_=src[:, c * 512 : min((c + 1) * 512, Ns)])
            nc.vector.max(out=dst8, in_=cands)

        def knn(qt4, Nq, rhs4, Ns, out_dram):
            nqt = Nq // P
            nchunk = (Ns + 511) // 512
            for j in range(nqt):
                lhsT = qt4[:, j * P : (j + 1) * P]
                dtile = knn_pool.tile([P, Ns], f32, tag="dtile")
                for c in range(nchunk):
                    cw = min(512, Ns - c * 512)
                    pmm = psum_mm.tile([P, 512], f32, tag="mm")
                    nc.tensor.matmul(
                        pmm[:, :cw], lhsT, rhs4[:, c * 512 : c * 512 + cw],
                        start=True, stop=True)
                    nc.scalar.activation(
                        dtile[:, c * 512 : c * 512 + cw], pmm[:, :cw], AF.Copy)
                # exact top-16: top-8, knock them out of a copy, top-8 again
                v1 = idx_pool.tile([P, 8], f32, tag="v1")
                v2 = idx_pool.tile([P, 8], f32, tag="v2")
                chunk_max8(v1, dtile, Ns, nchunk)
                d2 = d2_pool.tile([P, Ns], f32, tag="d2")
                nc.vector.match_replace(
                    out=d2, in_to_replace=v1, in_values=dtile, imm_value=NEG)
                chunk_max8(v2, d2, Ns, nchunk)
                i1 = idx_pool.tile([P, 8], u16, tag="i1")
                i2 = idx_pool.tile([P, 8], u16, tag="i2")
                nc.vector.max_index(out=i1, in_max=v1, in_values=dtile)
                nc.vector.max_index(out=i2, in_max=v2, in_values=d2)
                ii = idx_pool.tile([P, K], i32, tag="ii")
                nc.vector.tensor_copy(ii[:, 0:8], i1)
                nc.vector.tensor_copy(ii[:, 8:16], i2)
                nc.sync.dma_start(out_dram[j * P : (j + 1) * P, :], ii)

        if stage < 4:
            do_knn = False
        if not do_knn:
            iidummy = idx_pool.tile([P, K], i32, tag="ii")
            nc.vector.memset(iidummy, 0)
            for od, nq in ((n0o, N0 // 2), (u0o, N0 // 2), (s0o, N1), (n1o, N1), (u1o, N1), (s1o, N2), (n2o, N2)):
                for j in range(nq // P):
                    nc.sync.dma_start(od[j * P : (j + 1) * P, :], iidummy)
        else:
            knn(qt4_0, N0 // 2, rhs4_0, N0, n0o)   # knn0 half
            knn(qt4_0, N0 // 2, rhs4_1, N1, u0o)   # up0 half
            knn(qt4_1, N1, rhs4_0, N0, s0o)        # sub0 full
            knn(qt4_1, N1, rhs4_1, N1, n1o)        # knn1 full
            knn(qt4_1, N1, rhs4_2, N2, u1o)        # up1 full
            knn(qt4_2, N2, rhs4_1, N1, s1o)        # sub1 full
            knn(qt4_2, N2, rhs4_2, N2, n2o)        # knn2 full

    _split_ctrl_waits(nc)
    return nc


def _get_program():
    if "nc" not in _cache:
        _cache["nc"] = _build_program()
    return _cache["nc"]


def kernel(points: np.ndarray):
    from concourse.bass_utils import run_bass_kernel_spmd

    points = np.asarray(points)
    assert points.shape == (B, N0, 3)
    nc = _get_program()

    in_maps = []
    for core in range(NCORES):
        b, h = core // 2, core % 2
        pin = np.concatenate(
            [points[b], points[b, h * (N0 // 2) : (h + 1) * (N0 // 2)]], axis=0)
        in_maps.append({"pin": np.ascontiguousarray(pin, dtype=np.float32)})

    res = run_bass_kernel_spmd(nc, in_maps, list(range(NCORES)))

    def part(core, name):
        off, rows, cols = OLAYOUT[name]
        v = res.results[core]["oall"][off : off + rows * cols]
        if name in ("l1o", "l2o"):
            v = v.view(np.float32)
        return v.reshape(rows, cols)

    l1 = np.stack([part(2 * b, "l1o") for b in range(B)])
    l2 = np.stack([part(2 * b, "l2o") for b in range(B)])
    n0 = np.stack(
        [np.concatenate([part(2 * b, "n0o"), part(2 * b + 1, "n0o")], 0) for b in range(B)])
    u0 = np.stack(
        [np.concatenate([part(2 * b, "u0o"), part(2 * b + 1, "u0o")], 0) for b in range(B)])
    s0 = np.stack([part(2 * b, "s0o") for b in range(B)])
    n1 = np.stack([part(2 * b, "n1o") for b in range(B)])
    u1 = np.stack([part(2 * b, "u1o") for b in range(B)])
    s1 = np.stack([part(2 * b, "s1o") for b in range(B)])
    n2 = np.stack([part(2 * b, "n2o") for b in range(B)])

    return (
        points.astype(np.float32), l1, l2,
        n0.astype(np.int32), n1.astype(np.int32), n2.astype(np.int32),
        s0.astype(np.int32), s1.astype(np.int32),
        u0.astype(np.int32), u1.astype(np.int32),
    )
